# revision 15
# baseline (speedup 1.0000x reference)
"""Distorted-SSIM loss kernel for Trainium2 (8 NeuronCores, data parallel).

v3 — u-merged pointwise chain + unified PSUM ring.

Math per [512,512] plane pair (x, y):
    maps (host, fp32->fp16): w1 = x+y, w2 = x-y, up64 = 64(x^2+y^2),
    vp128 = 128*x*y.
    stage 1 (PE): column convs for both tapsets (g5, g11), image tile as
    the matmul stationary so the result lands transposed [w, h].
    stage 2 (PE): row convs at M=117 always (zero stationary cols beyond
    the valid outputs keep garbage rows at exactly 0); stationary row 127
    is a constant-1 bias row picking up 64*C2 from the cm tiles' row 127
    (full-width for U so garbage rows divide safely; width-clipped for V
    so garbage contributions are exactly 0).
    pointwise (ACT+DVE), scaled by 64 to keep fp16 normal, vectorized
    over all 5 w-windows per combo ([117, 5, *] tiles):
        sa|sb = (S|D * sqrt(32))^2       (ACT per-u, PSUM->SBUF)
        tp = sa-sb, q = sa+sb            (DVE TT, strided)
        f2|d2 = VU - (tp|q)              (DVE STT per-u, PSUM src)
        qC = q + 64*C1                   (DVE TS 4x)
        den = qC * d2                    (DVE TT)
        r = recip_approx_fast(den)       (DVE custom)
        g = f2 * r                       (DVE TT)
        s = (tp + 64*C1)*g; acc += sum   (DVE affine_mul_reduce)
    The 64/128 scales cancel exactly (powers of two).

Each core handles 4 images (12 planes); returns per-(plane,combo) sums
[128, 36] fp32; the host reduces.
"""

import sys
import numpy as np

for _p in ("/opt/trn_rl_repo",):
    if _p not in sys.path:
        sys.path.insert(0, _p)

SIGMA = 1.5
C1 = 0.01**2
C2 = 0.03**2
SC = 64.0

# h-direction (column conv) chunks: 128-row windows -> 118 outputs
HSTARTS = [0, 113, 231, 349, 467]
KSZH = [min(128, 512 - s) for s in HSTARTS]          # [128,128,128,128,45]
MSZH = [118, 118, 118, 118, 40]
# w-direction (row conv) windows: 127-row windows -> 117 outputs
# (row 127 of the stage-2 stationary is the bias row)
WOB = [0, 117, 234, 351, 468]                        # output bases
WSTARTS = [0, 112, 229, 346, 463]                    # include 5-left halo
KSZW = [min(127, 512 - s) for s in WSTARTS]          # [127,127,127,127,49]
MSZW = [117, 117, 117, 117, 44]

N_PLANES = 12
FREE = 5 * 512  # 2560 per map
COMBOS = [(0, 1), (1, 0), (1, 1)]  # (stage-1 tapset, stage-2 tapset); 0=g5 1=g11


def _gaussian(n, sigma=SIGMA):
    x = np.arange(n, dtype=np.float64)
    g = np.exp(-((x - n // 2) ** 2) / (2.0 * sigma**2))
    return (g / g.sum()).astype(np.float32)


def _norm_fp16_taps(g):
    """fp16 taps ULP-nudged so the fp64 sum is exactly 1.0."""
    t = g.astype(np.float16)
    for _ in range(500):
        td = t.astype(np.float64)
        err = td.sum() - 1.0
        if abs(err) < 2e-8:
            break
        bits = t.view(np.uint16).astype(np.int32) + (1 if err < 0 else -1)
        stepped = bits.astype(np.uint16).view(np.float16)
        delta = stepped.astype(np.float64) - td
        ad = np.abs(delta)
        ok = ad <= abs(err) * 1.000001
        i = int(np.argmax(np.where(ok, ad, -1.0))) if ok.any() else int(np.argmin(ad))
        t[i] = stepped[i]
    return t


def _wblocks_h():
    """Stage-1 banded conv blocks [128, 2, 5, 118]."""
    W = np.zeros((128, 2, 5, 118), np.float32)
    kk = np.arange(128)
    for t, k in ((0, 5), (1, 11)):
        g = _norm_fp16_taps(_gaussian(k)).astype(np.float32)
        p = k // 2
        for c, s in enumerate(HSTARTS):
            m = np.arange(MSZH[c])
            j = (s + kk[:, None]) - (118 * c + m[None, :]) + p
            valid = (j >= 0) & (j < k) & (kk[:, None] < KSZH[c])
            W[:, t, c, : MSZH[c]][valid] = g[np.clip(j, 0, k - 1)][valid]
    return W.astype(np.float16)


def _wblocks_w():
    """Stage-2 banded blocks [128, 2, 2, 5, 117]; dim 1: tapset, dim 2:
    bias variant (0: row127=1 full width, for S/D/U; 1: row127=1 only on
    valid outputs, for V)."""
    W = np.zeros((128, 2, 2, 5, 117), np.float32)
    kk = np.arange(127)
    for t, k in ((0, 5), (1, 11)):
        g = _norm_fp16_taps(_gaussian(k)).astype(np.float32)
        p = k // 2
        for u, s in enumerate(WSTARTS):
            m = np.arange(MSZW[u])
            j = (s + kk[:, None]) - (WOB[u] + m[None, :]) + p
            valid = (j >= 0) & (j < k) & (kk[:, None] < KSZW[u])
            blk = np.zeros((127, 117), np.float32)
            blk[:, : MSZW[u]][valid] = g[np.clip(j, 0, k - 1)][valid]
            W[:127, t, 0, u] = blk
            W[:127, t, 1, u] = blk
            W[127, t, 0, u, :] = 1.0
            W[127, t, 1, u, : MSZW[u]] = 1.0
    return W.astype(np.float16)


def _overlap_planes(pl):
    """[12, 512, 512] fp32 -> [12, 128, 2560] fp16 overlapped h-window tiles."""
    t = np.zeros((N_PLANES, 5, 128, 512), np.float32)
    for c, s in enumerate(HSTARTS):
        t[:, c, : KSZH[c], :] = pl[:, s : s + KSZH[c], :]
    return np.ascontiguousarray(
        t.transpose(0, 2, 1, 3).reshape(N_PLANES, 128, FREE)
    ).astype(np.float16)


_PROGRAM = {}


def _build_program():
    import concourse.bass as bass
    import concourse.mybir as mybir
    from concourse import bacc, tile
    from concourse.dve_ops import RECIP_APPROX_FAST_CONSTS, RECIPROCAL_APPROX_FAST

    f32 = mybir.dt.float32
    f16 = mybir.dt.float16
    Alu = mybir.AluOpType
    Act = mybir.ActivationFunctionType
    RC = RECIP_APPROX_FAST_CONSTS

    nc = bacc.Bacc(None, target_bir_lowering=False)
    m4_d = nc.dram_tensor("maps4", [N_PLANES, 128, 4 * FREE], f16, kind="ExternalInput")
    wb1_d = nc.dram_tensor("wb1", [128, 2, 5, 118], f16, kind="ExternalInput")
    wb2_d = nc.dram_tensor("wb2", [128, 2, 2, 5, 117], f16, kind="ExternalInput")
    gam_d = nc.dram_tensor("gamma", [1, 5 * 1024], f16, kind="ExternalInput")
    out_d = nc.dram_tensor("acc", [128, 36], f32, kind="ExternalOutput")

    SQS = float(np.sqrt(SC / 2.0))  # ACT square input scale -> 64*(S^2/2)
    C1S = SC * C1

    with tile.TileContext(nc) as tc:
        with (
            tc.tile_pool(name="const", bufs=1) as cpool,
            tc.tile_pool(name="m4", bufs=2) as mpool,
            tc.tile_pool(name="big", bufs=1) as bpool,
            tc.tile_pool(name="win", bufs=1) as wpool,
            tc.tile_pool(name="sb", bufs=4) as sbpool,
            tc.tile_pool(name="ps1", bufs=2, space="PSUM") as ps1pool,
            tc.tile_pool(name="psSD", bufs=1, space="PSUM") as psSDpool,
            tc.tile_pool(name="psVU", bufs=1, space="PSUM") as psVUpool,
        ):
            wb1 = cpool.tile([128, 2, 5, 118], f16, tag="wb1")
            wb2 = cpool.tile([128, 2, 2, 5, 117], f16, tag="wb2")
            nc.sync.dma_start(wb1[:], wb1_d[:])
            nc.sync.dma_start(wb2[:], wb2_d[:])
            acc = cpool.tile([128, 36], f32, tag="acc")
            nc.vector.memset(acc[:], 0.0)

            # cm tiles: per-map stage-1 results [w', u, t*512 + h]
            cms = []
            for mp in range(4):
                cm = cpool.tile([128, 5, 1024], f16, tag=f"cm{mp}")
                nc.vector.memset(cm[:], 0.0)
                if mp >= 2:  # up64, vp128 carry the C2 bias on row 127
                    nc.sync.dma_start(cm[127:128, :, :], gam_d[:])
                cms.append(cm)

            # dummy matmul: absorb the wb DMA wait on PE once
            dummy = ps1pool.tile([128, 1024], f32, tag="ps1")
            nc.tensor.matmul(
                dummy[0:118, 0:118], wb1[0:128, 0, 0, 0:118], wb1[0:128, 0, 0, 0:118],
                start=True, stop=True,
            )

            for p in range(N_PLANES):
                m4 = mpool.tile([128, 4 * FREE], f16, tag="m4")
                nc.sync.dma_start(m4[:], m4_d[p])

                fd = [bpool.tile([128, 5, 1024], f16, tag=f"fd{ci}",
                                 name=f"fd{ci}") for ci in range(3)]
                tpq = [bpool.tile([128, 5, 1024], f16, tag=f"tpq{ci}",
                                  name=f"tpq{ci}") for ci in range(3)]

                for u in range(5):
                    Kw = KSZW[u]
                    ws = WSTARTS[u]
                    # ---- stage 1: column convs for the 4 maps at w-window u
                    for mp in range(4):
                        ps = ps1pool.tile([128, 1024], f32, tag="ps1")
                        for t in (0, 1):
                            for c in range(5):
                                Kc, Mc = KSZH[c], MSZH[c]
                                base = 512 * t + 118 * c
                                nc.tensor.matmul(
                                    ps[0:Kw, base : base + Mc],
                                    m4[0:Kc, FREE * mp + 512 * c + ws :
                                       FREE * mp + 512 * c + ws + Kw],
                                    wb1[0:Kc, t, c, 0:Mc],
                                    start=True, stop=True,
                                )
                        nc.scalar.copy(cms[mp][0:Kw, u, :], ps[0:Kw, :])

                    # ---- stage 2 (M=117 always; garbage rows exactly 0)
                    for ci, (ct, rt) in enumerate(COMBOS):
                        SD = psSDpool.tile([128, 1024], f32, tag="sd")
                        VU = psVUpool.tile([128, 1024], f32, tag="vu")
                        for half, mp, pst, var in ((0, 0, SD, 0), (1, 1, SD, 0),
                                                   (0, 3, VU, 1), (1, 2, VU, 0)):
                            nc.tensor.matmul(
                                pst[0:117, 512 * half : 512 * half + 512],
                                wb2[0:128, rt, var, u, 0:117],
                                cms[mp][0:128, u, 512 * ct : 512 * ct + 512],
                                start=True, stop=True,
                            )
                        sasb = sbpool.tile([128, 1024], f16, tag="sasb")
                        nc.scalar.activation(
                            sasb[0:117, :], SD[0:117, :], Act.Square, scale=SQS
                        )
                        nc.vector.tensor_sub(
                            tpq[ci][0:117, u, 0:512],
                            sasb[0:117, 0:512], sasb[0:117, 512:1024],
                        )
                        nc.vector.tensor_add(
                            tpq[ci][0:117, u, 512:1024],
                            sasb[0:117, 0:512], sasb[0:117, 512:1024],
                        )
                        nc.vector.scalar_tensor_tensor(
                            fd[ci][0:117, u, :], VU[0:117, :], 1.0,
                            tpq[ci][0:117, u, :], op0=Alu.mult, op1=Alu.subtract,
                        )

                # ---- merged pointwise tail per combo over all 5 u-windows
                for ci in range(3):
                    tp_ap = tpq[ci][0:117, :, 0:512]
                    q_ap = tpq[ci][0:117, :, 512:1024]
                    f2_ap = fd[ci][0:117, :, 0:512]
                    d2_ap = fd[ci][0:117, :, 512:1024]
                    qC = wpool.tile([128, 5, 512], f16, tag="qC")
                    nc.scalar.activation(qC[0:117], q_ap, Act.Copy, bias=C1S)
                    den = wpool.tile([128, 5, 512], f16, tag="den")
                    nc.vector.tensor_tensor(den[0:117], qC[0:117], d2_ap, Alu.mult)
                    r = wpool.tile([128, 5, 512], f16, tag="r")
                    if ci == 1:
                        # offload one combo's reciprocal to ScalarE: 1/x = exp(-ln x)
                        rln = wpool.tile([128, 5, 512], f16, tag="rln")
                        nc.scalar.activation(rln[0:117], den[0:117], Act.Ln)
                        nc.scalar.activation(r[0:117], rln[0:117], Act.Exp, scale=-1.0)
                    else:
                        nc.vector._custom_dve(
                            RECIPROCAL_APPROX_FAST, out=r[0:117], in0=den[0:117],
                            s0=RC["s0"], s1=RC["s1"], imm2=RC["imm2"],
                        )
                    g = wpool.tile([128, 5, 512], f16, tag="g")
                    nc.vector.tensor_tensor(g[0:117], f2_ap, r[0:117], Alu.mult)
                    scr = wpool.tile([128, 5, 512], f16, tag="scr")
                    col = p * 3 + ci
                    nc.vector.affine_mul_reduce(
                        out=scr[0:117], accum_out=acc[0:117, col : col + 1],
                        in0=tp_ap, in1=g[0:117],
                        scale=1.0, bias=C1S,
                    )

            nc.sync.dma_start(out_d[:], acc[:])

    nc.finalize()
    return nc


def _get_program():
    global _PROGRAM
    if not isinstance(_PROGRAM, dict):
        globals()["_PROGRAM"] = {}
    if "v3" not in _PROGRAM:
        _PROGRAM["v3"] = _build_program()
    return _PROGRAM["v3"]


def _make_in_maps(img1, img2):
    x = np.asarray(img1)[:, :3].astype(np.float32)
    y = np.asarray(img2)[:, :3].astype(np.float32)
    wb1 = _wblocks_h()
    wb2 = _wblocks_w()
    gamma = np.full((1, 5 * 1024), SC * C2, np.float16)
    in_maps = []
    for i in range(8):
        xs = x[4 * i : 4 * i + 4].reshape(N_PLANES, 512, 512)
        ys = y[4 * i : 4 * i + 4].reshape(N_PLANES, 512, 512)
        w1 = _overlap_planes(xs + ys)
        w2 = _overlap_planes(xs - ys)
        up = _overlap_planes(SC * (xs * xs + ys * ys))
        vp = _overlap_planes((2.0 * SC) * (xs * ys))
        m4 = np.concatenate([w1, w2, up, vp], axis=2)  # [12, 128, 4*2560]
        in_maps.append({"maps4": m4, "wb1": wb1, "wb2": wb2, "gamma": gamma})
    return in_maps


def _reduce_results(res):
    total = 0.0
    for i in range(8):
        total += np.asarray(res[i]["acc"]).astype(np.float64).sum()
    npix = 32 * 3 * 512 * 512
    return np.float32(total / npix / 3.0)


def kernel(img1, img2):
    from concourse.bass_utils import run_bass_kernel_spmd

    in_maps = _make_in_maps(img1, img2)
    nc = _get_program()
    res = run_bass_kernel_spmd(nc, in_maps, core_ids=list(range(8))).results
    return _reduce_results(res)
